# revision 42
# baseline (speedup 1.0000x reference)
"""Trainium2 Bass kernel for nn_AutoencoderHybrid_65481071408310.

Math: the reference simulates an 8-qubit circuit per sample. The RX-encoding
layer produces a product state whose amplitudes factor as
    psi[k] = m[k] * (-i)^popcount(k),   m[k] = prod_i (cos(x_i/2) or sin(x_i/2))
and the StronglyEntanglingLayers form a fixed 256x256 unitary U depending only
on q_weights.  Folding the popcount phases into U gives a REAL matmul
    phi = m @ M,  M = [Re(W) | Im(W)],  W = (U * (-i)^popcount)^T
then probs_k = Re_k^2 + Im_k^2, q_j = probs @ (signs @ w1.T), relu, w2 head.

Device pipeline per core (8192 samples, fp16 matmul operands):
  ACT cos/sin -> DVE 3-level outer-product tree in NATURAL layout (samples on
  partitions, stride-0 broadcast APs, zero DMA) -> DMA XBAR transpose to get
  mT (k on partitions, kh-major so phi rhs is contiguous) -> PE phi matmuls
  (channel layout [Re_k | Im_k] on the same partition) -> squares on ACT ->
  PE A-contract: 4 accumulating matmuls into one (4,512) PSUM tile (the PSUM
  accumulation performs the Re^2+Im^2 pair-sum for free) -> DVE bias+relu ->
  PE head matmuls -> DVE copy (+b2 via stride-0 broadcast) -> DMA out.
  The PE queue is software-pipelined: phi(i), A(i-1), head(i-2) so
  cross-engine latency never stalls it; staging runs one ctile ahead
  (ctile 0 in 4-chunk pieces for a short lead-in).
"""
import sys
import numpy as np

sys.path.insert(0, '/opt/trn_rl_repo')

import concourse.bacc as bacc
import concourse.mybir as mybir
import concourse.tile as tile
from concourse.bass_utils import run_bass_kernel_spmd

F32 = mybir.dt.float32
F16 = mybir.dt.float16
AFT = mybir.ActivationFunctionType
ALU = mybir.AluOpType

NQ = 8
DIM = 256
REPS = 4
INPUT_DIM = 8
LATENT = 4
BATCH = 65536
NCORES = 8
BC = BATCH // NCORES          # 8192 samples per core
NCHUNK = 64                   # 64 chunks of 128 samples; sample = 64*p + n
NCTILE = 4                    # 4 ctiles of 16 chunks
NBLK = 16                     # 16 blocks of 512 samples (4 per ctile)

LAST_RESULTS = None           # test harness introspection


# ---------------------------------------------------------------- host math
def _rot_mat(phi, theta, omega):
    c, s = np.cos(theta / 2), np.sin(theta / 2)
    return np.array([
        [np.exp(-0.5j * (phi + omega)) * c, -np.exp(0.5j * (phi - omega)) * s],
        [np.exp(-0.5j * (phi - omega)) * s, np.exp(0.5j * (phi + omega)) * c],
    ], dtype=np.complex128)


def _kron_list(ops):
    full = ops[0]
    for o in ops[1:]:
        full = np.kron(full, o)
    return full


def _build_entangler(qw):
    I2 = np.eye(2, dtype=np.complex128)
    P0 = np.array([[1, 0], [0, 0]], dtype=np.complex128)
    P1 = np.array([[0, 0], [0, 1]], dtype=np.complex128)
    X = np.array([[0, 1], [1, 0]], dtype=np.complex128)
    U = np.eye(DIM, dtype=np.complex128)
    for l in range(REPS):
        for i in range(NQ):
            ops = [I2] * NQ
            ops[i] = _rot_mat(*qw[l, i])
            U = _kron_list(ops) @ U
        r = (l % (NQ - 1)) + 1
        for i in range(NQ):
            t = (i + r) % NQ
            ops0 = [I2] * NQ
            ops0[i] = P0
            ops1 = [I2] * NQ
            ops1[i] = P1
            ops1[t] = X
            U = (_kron_list(ops0) + _kron_list(ops1)) @ U
    return U


def _host_consts(q_weights, w1, b1, w2, b2):
    U = _build_entangler(q_weights.astype(np.float64))
    pop = np.array([bin(k).count('1') for k in range(DIM)])
    W = (U * ((-1j) ** pop)[None, :]).T          # phi = m @ W   (256 x 256 cplx)
    ks = np.arange(DIM)
    signs = 1.0 - 2.0 * ((ks[:, None] >> (NQ - 1 - np.arange(NQ))[None, :]) & 1)
    A = signs @ w1.T.astype(np.float64)           # (256, 4)

    # vmat[p', 4*kh+2*jp+e, p] = (e==0 ? ReW : ImW)[128*kh+p', 128*jp+p]
    vmat = np.zeros((128, 8, 128), np.float64)
    for kh in range(2):
        for jp in range(2):
            for e in range(2):
                src = W.real if e == 0 else W.imag
                vmat[:, 4 * kh + 2 * jp + e, :] = \
                    src[128 * kh:128 * (kh + 1), 128 * jp:128 * (jp + 1)]
    # amat[p, 4*jp + j] = A[128*jp + p, j]
    amat = np.zeros((128, 8), np.float64)
    for jp in range(2):
        amat[:, 4 * jp:4 * jp + 4] = A[128 * jp:128 * (jp + 1), :]
    return {
        'vmat': np.ascontiguousarray(vmat.reshape(128, 1024).astype(np.float16)),
        'amat': np.ascontiguousarray(amat.astype(np.float16)),
        'w2s': np.ascontiguousarray(w2.T.astype(np.float16)),
        'b1c': np.ascontiguousarray(b1.astype(np.float32).reshape(LATENT, 1)),
        'b2b': np.ascontiguousarray(
            np.broadcast_to(b2.astype(np.float32), (128, INPUT_DIM)).copy()),
    }


# ---------------------------------------------------------------- bass build
def _build_nc():
    nc = bacc.Bacc(None, target_bir_lowering=False)
    xs = nc.declare_dram_parameter("xs", [BC, INPUT_DIM], F32, isOutput=False)
    vmat = nc.declare_dram_parameter("vmat", [128, 1024], F16, isOutput=False)
    amat = nc.declare_dram_parameter("amat", [128, 8], F16, isOutput=False)
    w2s = nc.declare_dram_parameter("w2s", [LATENT, INPUT_DIM], F16, isOutput=False)
    b1c = nc.declare_dram_parameter("b1c", [LATENT, 1], F32, isOutput=False)
    b2b = nc.declare_dram_parameter("b2b", [128, INPUT_DIM], F32, isOutput=False)
    out = nc.declare_dram_parameter("out", [BC, INPUT_DIM], F32, isOutput=True)

    with tile.TileContext(nc) as tc:
        with (
            tc.tile_pool(name="const", bufs=1) as cst,
            tc.tile_pool(name="stage", bufs=1) as stg,
            tc.tile_pool(name="sq", bufs=2) as sqp,
            tc.tile_pool(name="hh", bufs=2) as hhp,
            tc.tile_pool(name="phip", bufs=1, space="PSUM") as phip,
            tc.tile_pool(name="qp", bufs=2, space="PSUM") as qpp,
            tc.tile_pool(name="wp", bufs=2, space="PSUM") as wpp,
        ):
            # ---- constants + input
            zero = cst.tile([128, 1], F32)
            nc.vector.memset(zero[:], 0.0)
            halfpi = cst.tile([128, 1], F32)
            nc.vector.memset(halfpi[:], float(np.pi / 2))
            warm = cst.tile([1, 2], F16)
            nc.scalar.activation(warm[:, 0:1], zero[0:1, :], AFT.Sin,
                                 scale=1.0, bias=zero[0:1, :])
            xnat = stg.tile([128, NCHUNK, INPUT_DIM], F32)   # sample = 64p + n
            xsr = xs.rearrange("(p n) d -> p n d", n=NCHUNK)
            nc.sync.dma_start(xnat[:, 0:4, :], xsr[:, 0:4, :])
            vt = cst.tile([128, 8, 128], F16)
            nc.sync.dma_start(vt[:], vmat.rearrange("p (g c) -> p g c", g=8))
            nc.gpsimd.dma_start(xnat[:, 4:NCHUNK, :], xsr[:, 4:NCHUNK, :])
            at = cst.tile([128, 8], F16)
            nc.gpsimd.dma_start(at[:], amat[:])
            w2t = cst.tile([LATENT, INPUT_DIM], F16)
            nc.gpsimd.dma_start(w2t[:], w2s[:])
            b1t = cst.tile([LATENT, 1], F32)
            nc.gpsimd.dma_start(b1t[:], b1c[:])
            b2t = cst.tile([128, INPUT_DIM], F32)
            nc.gpsimd.dma_start(b2t[:], b2b[:])

            # ---- persistent staging tiles (subtile deps per piece)
            csnat = stg.tile([128, NCHUNK, 16], F16)    # col 16n + 2w + b
            ptile = stg.tile([128, NCHUNK, 16], F16)    # col 16n + 4q + 2b0+b1
            hl = stg.tile([128, NCHUNK, 32], F16)       # col 32n + 16h + 4c0+c1
            # kh-major m: [kh, n, hi', lo] with k = 128*kh + 16*hi' + lo
            mnat = stg.tile([128, 2, NCHUNK, 8, 16], F16)
            mtt = stg.tile([128, 2, NCHUNK, 128], F16)  # [kp, kh, n, p]
            onat = stg.tile([128, NCHUNK, INPUT_DIM], F32)

            def stage_tree(c0, nch):
                ns = slice(c0, c0 + nch)
                xg = xnat[:, ns, :]
                cs5 = csnat.rearrange("p n (w b) -> p n w b", w=8)
                nc.scalar.activation(cs5[:, ns, :, 0], xg, AFT.Sin,
                                     scale=0.5, bias=halfpi[:])
                nc.scalar.activation(cs5[:, ns, :, 1], xg, AFT.Sin,
                                     scale=0.5, bias=zero[:])
                # L1: pair products; in0=f(2q,b0) bcast b1, in1=f(2q+1,b1) bcast b0
                c6 = csnat.rearrange("p n (q y x) -> p n q y x", q=4, y=2)
                in0 = c6[:, ns, :, 0, :].unsqueeze(4).broadcast_to(
                    [128, nch, 4, 2, 2])
                in1 = c6[:, ns, :, 1, :].unsqueeze(3).broadcast_to(
                    [128, nch, 4, 2, 2])
                nc.vector.tensor_mul(
                    ptile.rearrange("p n (q b0 b1) -> p n q b0 b1",
                                    q=4, b0=2)[:, ns],
                    in0, in1)
                # L2: hi/lo; h=0: q0 x q1, h=1: q2 x q3
                p6 = ptile.rearrange("p n (h q c) -> p n h q c", h=2, q=2)
                j0 = p6[:, ns, :, 0, :].unsqueeze(4).broadcast_to(
                    [128, nch, 2, 4, 4])
                j1 = p6[:, ns, :, 1, :].unsqueeze(3).broadcast_to(
                    [128, nch, 2, 4, 4])
                nc.vector.tensor_mul(
                    hl.rearrange("p n (h c0 c1) -> p n h c0 c1",
                                 h=2, c0=4)[:, ns],
                    j0, j1)
                # L3 split by kh: m[kh, n, hi', lo] = hl[hi=8kh+hi'] * hl[16+lo]
                h6 = hl.rearrange("p n (h c) -> p n h c", h=2)
                for kh in range(2):
                    k0 = h6[:, ns, 0, 8 * kh:8 * kh + 8].unsqueeze(3) \
                        .broadcast_to([128, nch, 8, 16])
                    k1 = h6[:, ns, 1, :].unsqueeze(2) \
                        .broadcast_to([128, nch, 8, 16])
                    nc.vector.tensor_mul(mnat[:, kh, ns, :, :], k0, k1)

            def stage_transpose(c0, nch, split=False):
                # XBAR transpose per kh: (128, nch*128) -> (128, nch, 128)
                for kh in range(2):
                    eng = nc.scalar if (split and kh == 0) else nc.sync
                    eng.dma_start(
                        mtt[:, kh, c0:c0 + nch, :],
                        mnat[:, kh, c0:c0 + nch, :, :].rearrange(
                            "p n h l -> p (n h l)"),
                        transpose=True)

            phis = {}
            probs = {}
            qs = {}
            h5s = {}
            wns = {}

            def emit_phi(i):
                n0 = 4 * i
                phi = [None, None]
                for jp in range(2):
                    ph = phip.tile([128, 1024], F32, tag=f"phi{jp}")
                    for e in range(2):
                        for kh in range(2):
                            rhs = mtt[:, kh, n0:n0 + 4, :]
                            nc.tensor.matmul(
                                ph[:, 512 * e:512 * (e + 1)],
                                vt[:, 4 * kh + 2 * jp + e, :], rhs,
                                start=(kh == 0), stop=(kh == 1))
                    phi[jp] = ph
                phis[i] = phi
                # squares on ACT (single-PSUM-operand rule); last blocks split
                # per e-half so the drain's A-matmuls start sooner
                pr = [None, None]
                for jp in range(2):
                    sq = sqp.tile([128, 1024], F16, tag=f"sq{jp}")
                    if i >= NBLK - 2:
                        # drain blocks: per-half squares so each A-matmul
                        # starts after 512 cols instead of 1024
                        for e in range(2):
                            nc.scalar.activation(
                                sq[:, 512 * e:512 * (e + 1)],
                                phi[jp][:, 512 * e:512 * (e + 1)], AFT.Square)
                    else:
                        nc.scalar.activation(sq[:], phi[jp][:], AFT.Square)
                    pr[jp] = sq
                probs[i] = pr

            def emit_contract(i):
                pr = probs.pop(i)
                q = qpp.tile([LATENT, 512], F32, tag="q")
                # 4 accumulating matmuls: PSUM does the Re^2+Im^2 pair-sum
                for jp in range(2):
                    for e in range(2):
                        nc.tensor.matmul(
                            q[:], at[:, 4 * jp:4 * jp + 4],
                            pr[jp][:, 512 * e:512 * (e + 1)],
                            start=(jp == 0 and e == 0),
                            stop=(jp == 1 and e == 1))
                h5 = hhp.tile([LATENT, 512], F16, tag="h5")
                nc.vector.tensor_scalar(h5[:], q[:], b1t[:], 0.0,
                                        ALU.add, ALU.max)
                qs[i] = q
                h5s[i] = h5
                del phis[i]

            def emit_head(i):
                g, bl = i // 4, i % 4
                h5 = h5s.pop(i)
                qs.pop(i)
                wn = wpp.tile([128, 4, INPUT_DIM], F32, tag="wn")
                for c in range(4):
                    nc.tensor.matmul(wn[:, c, :], h5[:, 128 * c:128 * (c + 1)],
                                     w2t[:], start=True, stop=True)
                n0 = 16 * g + 4 * bl
                nc.vector.tensor_add(
                    onat[:, n0:n0 + 4, :], wn[:],
                    b2t.unsqueeze(1).broadcast_to([128, 4, INPUT_DIM]))
                outr = out.rearrange("(p n) d -> p n d", n=NCHUNK)
                if g == NCTILE - 1:
                    # last ctile: per-block DMA on hwdge (gpsimd's software
                    # DGE issue + end-of-program drain sat on the tail)
                    nc.scalar.dma_start(outr[:, n0:n0 + 4, :],
                                        onat[:, n0:n0 + 4, :])
                elif bl == 3:
                    nc.scalar.dma_start(
                        outr[:, 16 * g:16 * (g + 1), :],
                        onat[:, 16 * g:16 * (g + 1), :])

            # ctile 0 staged in 4-chunk pieces for a short lead-in; later
            # ctiles staged whole, one ctile ahead of the PE.
            stage_tree(0, 4)
            stage_transpose(0, 4)
            # Square table loads here, off the first-piece critical chain
            nc.scalar.activation(warm[:, 1:2], zero[0:1, :], AFT.Square)
            SCHED = {0: (4, 4), 1: (8, 4), 2: (12, 4),
                     3: (16, 16), 6: (32, 16), 10: (48, 16)}
            for i in range(NBLK + 2):
                if i < NBLK:
                    emit_phi(i)
                    if i in SCHED:
                        stage_tree(*SCHED[i])
                        stage_transpose(*SCHED[i])
                if 1 <= i <= NBLK:
                    emit_contract(i - 1)
                if i >= 2:
                    emit_head(i - 2)

    nc.compile()
    return nc


_NC_CACHE = []


def _get_nc():
    if not _NC_CACHE:
        _NC_CACHE.append(_build_nc())
    return _NC_CACHE[0]


def kernel(x, q_weights, w1, b1, w2, b2):
    global LAST_RESULTS
    x = np.ascontiguousarray(np.asarray(x, dtype=np.float32))
    consts = _host_consts(np.asarray(q_weights), np.asarray(w1),
                          np.asarray(b1), np.asarray(w2), np.asarray(b2))
    nc = _get_nc()
    in_maps = [
        {'xs': np.ascontiguousarray(x[i * BC:(i + 1) * BC]), **consts}
        for i in range(NCORES)
    ]
    res = run_bass_kernel_spmd(nc, in_maps, list(range(NCORES)))
    LAST_RESULTS = res
    return np.concatenate([res.results[i]['out'] for i in range(NCORES)],
                          axis=0).astype(np.float32)


# revision 43
# speedup vs baseline: 1.1816x; 1.1816x over previous
"""Trainium2 Bass kernel for nn_AutoencoderHybrid_65481071408310.

Math: the reference simulates an 8-qubit circuit per sample. The RX-encoding
layer produces a product state whose amplitudes factor as
    psi[k] = m[k] * (-i)^popcount(k),   m[k] = prod_i (cos(x_i/2) or sin(x_i/2))
and the StronglyEntanglingLayers form a fixed 256x256 unitary U depending only
on q_weights.  Folding the popcount phases into U gives a REAL matmul
    phi = m @ M,  M = [Re(W) | Im(W)],  W = (U * (-i)^popcount)^T
then probs_k = Re_k^2 + Im_k^2, q_j = probs @ (signs @ w1.T), relu, w2 head.

Device pipeline per core (8192 samples, fp16 matmul operands):
  ACT cos/sin -> DVE 3-level outer-product tree in NATURAL layout (samples on
  partitions, stride-0 broadcast APs, zero DMA) -> DMA XBAR transpose to get
  mT (k on partitions, kh-major so phi rhs is contiguous) -> PE phi matmuls
  (channel layout [Re_k | Im_k] on the same partition) -> squares on ACT ->
  PE A-contract: 4 accumulating matmuls into one (4,512) PSUM tile (the PSUM
  accumulation performs the Re^2+Im^2 pair-sum for free) -> DVE bias+relu ->
  PE head matmuls -> DVE copy (+b2 via stride-0 broadcast) -> DMA out.
  The PE queue is software-pipelined: phi(i), A(i-1), head(i-2) so
  cross-engine latency never stalls it; staging runs one ctile ahead
  (ctile 0 in 4-chunk pieces for a short lead-in).
"""
import sys
import numpy as np

sys.path.insert(0, '/opt/trn_rl_repo')

import concourse.bacc as bacc
import concourse.mybir as mybir
import concourse.tile as tile
from concourse.bass_utils import run_bass_kernel_spmd

F32 = mybir.dt.float32
F16 = mybir.dt.float16
AFT = mybir.ActivationFunctionType
ALU = mybir.AluOpType

NQ = 8
DIM = 256
REPS = 4
INPUT_DIM = 8
LATENT = 4
BATCH = 65536
NCORES = 8
BC = BATCH // NCORES          # 8192 samples per core
NCHUNK = 64                   # 64 chunks of 128 samples; sample = 64*p + n
NCTILE = 4                    # 4 ctiles of 16 chunks
NBLK = 16                     # 16 blocks of 512 samples (4 per ctile)

LAST_RESULTS = None           # test harness introspection


# ---------------------------------------------------------------- host math
def _rot_mat(phi, theta, omega):
    c, s = np.cos(theta / 2), np.sin(theta / 2)
    return np.array([
        [np.exp(-0.5j * (phi + omega)) * c, -np.exp(0.5j * (phi - omega)) * s],
        [np.exp(-0.5j * (phi - omega)) * s, np.exp(0.5j * (phi + omega)) * c],
    ], dtype=np.complex128)


def _kron_list(ops):
    full = ops[0]
    for o in ops[1:]:
        full = np.kron(full, o)
    return full


def _build_entangler(qw):
    I2 = np.eye(2, dtype=np.complex128)
    P0 = np.array([[1, 0], [0, 0]], dtype=np.complex128)
    P1 = np.array([[0, 0], [0, 1]], dtype=np.complex128)
    X = np.array([[0, 1], [1, 0]], dtype=np.complex128)
    U = np.eye(DIM, dtype=np.complex128)
    for l in range(REPS):
        for i in range(NQ):
            ops = [I2] * NQ
            ops[i] = _rot_mat(*qw[l, i])
            U = _kron_list(ops) @ U
        r = (l % (NQ - 1)) + 1
        for i in range(NQ):
            t = (i + r) % NQ
            ops0 = [I2] * NQ
            ops0[i] = P0
            ops1 = [I2] * NQ
            ops1[i] = P1
            ops1[t] = X
            U = (_kron_list(ops0) + _kron_list(ops1)) @ U
    return U


def _host_consts(q_weights, w1, b1, w2, b2):
    U = _build_entangler(q_weights.astype(np.float64))
    pop = np.array([bin(k).count('1') for k in range(DIM)])
    W = (U * ((-1j) ** pop)[None, :]).T          # phi = m @ W   (256 x 256 cplx)
    ks = np.arange(DIM)
    signs = 1.0 - 2.0 * ((ks[:, None] >> (NQ - 1 - np.arange(NQ))[None, :]) & 1)
    A = signs @ w1.T.astype(np.float64)           # (256, 4)

    # vmat[p', 4*kh+2*jp+e, p] = (e==0 ? ReW : ImW)[128*kh+p', 128*jp+p]
    vmat = np.zeros((128, 8, 128), np.float64)
    for kh in range(2):
        for jp in range(2):
            for e in range(2):
                src = W.real if e == 0 else W.imag
                vmat[:, 4 * kh + 2 * jp + e, :] = \
                    src[128 * kh:128 * (kh + 1), 128 * jp:128 * (jp + 1)]
    # amat[p, 4*jp + j] = A[128*jp + p, j]
    amat = np.zeros((128, 8), np.float64)
    for jp in range(2):
        amat[:, 4 * jp:4 * jp + 4] = A[128 * jp:128 * (jp + 1), :]
    return {
        'vmat': np.ascontiguousarray(vmat.reshape(128, 1024).astype(np.float16)),
        'amat': np.ascontiguousarray(amat.astype(np.float16)),
        'w2s': np.ascontiguousarray(w2.T.astype(np.float16)),
        'b1c': np.ascontiguousarray(b1.astype(np.float32).reshape(LATENT, 1)),
        'b2b': np.ascontiguousarray(
            np.broadcast_to(b2.astype(np.float32), (128, INPUT_DIM)).copy()),
    }


# ---------------------------------------------------------------- bass build
def _build_nc():
    nc = bacc.Bacc(None, target_bir_lowering=False)
    xs = nc.declare_dram_parameter("xs", [BC, INPUT_DIM], F32, isOutput=False)
    vmat = nc.declare_dram_parameter("vmat", [128, 1024], F16, isOutput=False)
    amat = nc.declare_dram_parameter("amat", [128, 8], F16, isOutput=False)
    w2s = nc.declare_dram_parameter("w2s", [LATENT, INPUT_DIM], F16, isOutput=False)
    b1c = nc.declare_dram_parameter("b1c", [LATENT, 1], F32, isOutput=False)
    b2b = nc.declare_dram_parameter("b2b", [128, INPUT_DIM], F32, isOutput=False)
    out = nc.declare_dram_parameter("out", [BC, INPUT_DIM], F32, isOutput=True)

    with tile.TileContext(nc) as tc:
        with (
            tc.tile_pool(name="const", bufs=1) as cst,
            tc.tile_pool(name="stage", bufs=1) as stg,
            tc.tile_pool(name="sq", bufs=2) as sqp,
            tc.tile_pool(name="hh", bufs=2) as hhp,
            tc.tile_pool(name="phip", bufs=1, space="PSUM") as phip,
            tc.tile_pool(name="qp", bufs=2, space="PSUM") as qpp,
            tc.tile_pool(name="wp", bufs=2, space="PSUM") as wpp,
        ):
            # ---- constants + input
            zero = cst.tile([128, 1], F32)
            nc.vector.memset(zero[:], 0.0)
            halfpi = cst.tile([128, 1], F32)
            nc.vector.memset(halfpi[:], float(np.pi / 2))
            warm = cst.tile([1, 2], F16)
            nc.scalar.activation(warm[:, 0:1], zero[0:1, :], AFT.Sin,
                                 scale=1.0, bias=zero[0:1, :])
            xnat = stg.tile([128, NCHUNK, INPUT_DIM], F32)   # sample = 64p + n
            xsr = xs.rearrange("(p n) d -> p n d", n=NCHUNK)
            nc.sync.dma_start(xnat[:, 0:4, :], xsr[:, 0:4, :])
            vt = cst.tile([128, 8, 128], F16)
            nc.sync.dma_start(vt[:], vmat.rearrange("p (g c) -> p g c", g=8))
            nc.gpsimd.dma_start(xnat[:, 4:NCHUNK, :], xsr[:, 4:NCHUNK, :])
            at = cst.tile([128, 8], F16)
            nc.gpsimd.dma_start(at[:], amat[:])
            w2t = cst.tile([LATENT, INPUT_DIM], F16)
            nc.gpsimd.dma_start(w2t[:], w2s[:])
            b1t = cst.tile([LATENT, 1], F32)
            nc.gpsimd.dma_start(b1t[:], b1c[:])
            b2t = cst.tile([128, INPUT_DIM], F32)
            nc.gpsimd.dma_start(b2t[:], b2b[:])

            # ---- persistent staging tiles (subtile deps per piece)
            csnat = stg.tile([128, NCHUNK, 16], F16)    # col 16n + 2w + b
            ptile = stg.tile([128, NCHUNK, 16], F16)    # col 16n + 4q + 2b0+b1
            hl = stg.tile([128, NCHUNK, 32], F16)       # col 32n + 16h + 4c0+c1
            # kh-major m: [kh, n, hi', lo] with k = 128*kh + 16*hi' + lo
            mnat = stg.tile([128, 2, NCHUNK, 8, 16], F16)
            mtt = stg.tile([128, 2, NCHUNK, 128], F16)  # [kp, kh, n, p]
            onat = stg.tile([128, NCHUNK, INPUT_DIM], F32)

            def stage_tree(c0, nch):
                ns = slice(c0, c0 + nch)
                xg = xnat[:, ns, :]
                cs5 = csnat.rearrange("p n (w b) -> p n w b", w=8)
                nc.scalar.activation(cs5[:, ns, :, 0], xg, AFT.Sin,
                                     scale=0.5, bias=halfpi[:])
                nc.scalar.activation(cs5[:, ns, :, 1], xg, AFT.Sin,
                                     scale=0.5, bias=zero[:])
                # L1: pair products; in0=f(2q,b0) bcast b1, in1=f(2q+1,b1) bcast b0
                c6 = csnat.rearrange("p n (q y x) -> p n q y x", q=4, y=2)
                in0 = c6[:, ns, :, 0, :].unsqueeze(4).broadcast_to(
                    [128, nch, 4, 2, 2])
                in1 = c6[:, ns, :, 1, :].unsqueeze(3).broadcast_to(
                    [128, nch, 4, 2, 2])
                nc.vector.tensor_mul(
                    ptile.rearrange("p n (q b0 b1) -> p n q b0 b1",
                                    q=4, b0=2)[:, ns],
                    in0, in1)
                # L2: hi/lo; h=0: q0 x q1, h=1: q2 x q3
                p6 = ptile.rearrange("p n (h q c) -> p n h q c", h=2, q=2)
                j0 = p6[:, ns, :, 0, :].unsqueeze(4).broadcast_to(
                    [128, nch, 2, 4, 4])
                j1 = p6[:, ns, :, 1, :].unsqueeze(3).broadcast_to(
                    [128, nch, 2, 4, 4])
                nc.vector.tensor_mul(
                    hl.rearrange("p n (h c0 c1) -> p n h c0 c1",
                                 h=2, c0=4)[:, ns],
                    j0, j1)
                # L3 split by kh: m[kh, n, hi', lo] = hl[hi=8kh+hi'] * hl[16+lo]
                h6 = hl.rearrange("p n (h c) -> p n h c", h=2)
                for kh in range(2):
                    k0 = h6[:, ns, 0, 8 * kh:8 * kh + 8].unsqueeze(3) \
                        .broadcast_to([128, nch, 8, 16])
                    k1 = h6[:, ns, 1, :].unsqueeze(2) \
                        .broadcast_to([128, nch, 8, 16])
                    nc.vector.tensor_mul(mnat[:, kh, ns, :, :], k0, k1)

            def stage_transpose(c0, nch, split=False):
                # XBAR transpose per kh: (128, nch*128) -> (128, nch, 128)
                for kh in range(2):
                    eng = nc.scalar if (split and kh == 0) else nc.sync
                    eng.dma_start(
                        mtt[:, kh, c0:c0 + nch, :],
                        mnat[:, kh, c0:c0 + nch, :, :].rearrange(
                            "p n h l -> p (n h l)"),
                        transpose=True)

            phis = {}
            probs = {}
            qs = {}
            h5s = {}
            wns = {}

            def emit_phi(i):
                n0 = 4 * i
                phi = [None, None]
                for jp in range(2):
                    ph = phip.tile([128, 1024], F32, tag=f"phi{jp}")
                    for e in range(2):
                        for kh in range(2):
                            rhs = mtt[:, kh, n0:n0 + 4, :]
                            nc.tensor.matmul(
                                ph[:, 512 * e:512 * (e + 1)],
                                vt[:, 4 * kh + 2 * jp + e, :], rhs,
                                start=(kh == 0), stop=(kh == 1))
                    phi[jp] = ph
                phis[i] = phi
                # squares on ACT (single-PSUM-operand rule); last blocks split
                # per e-half so the drain's A-matmuls start sooner
                pr = [None, None]
                for jp in range(2):
                    sq = sqp.tile([128, 1024], F16, tag=f"sq{jp}")
                    nc.scalar.activation(sq[:], phi[jp][:], AFT.Square)
                    pr[jp] = sq
                probs[i] = pr

            def emit_contract(i):
                pr = probs.pop(i)
                q = qpp.tile([LATENT, 512], F32, tag="q")
                # 4 accumulating matmuls: PSUM does the Re^2+Im^2 pair-sum
                for jp in range(2):
                    for e in range(2):
                        nc.tensor.matmul(
                            q[:], at[:, 4 * jp:4 * jp + 4],
                            pr[jp][:, 512 * e:512 * (e + 1)],
                            start=(jp == 0 and e == 0),
                            stop=(jp == 1 and e == 1))
                h5 = hhp.tile([LATENT, 512], F16, tag="h5")
                nc.vector.tensor_scalar(h5[:], q[:], b1t[:], 0.0,
                                        ALU.add, ALU.max)
                qs[i] = q
                h5s[i] = h5
                del phis[i]

            def emit_head(i):
                g, bl = i // 4, i % 4
                h5 = h5s.pop(i)
                qs.pop(i)
                wn = wpp.tile([128, 4, INPUT_DIM], F32, tag="wn")
                for c in range(4):
                    nc.tensor.matmul(wn[:, c, :], h5[:, 128 * c:128 * (c + 1)],
                                     w2t[:], start=True, stop=True)
                n0 = 16 * g + 4 * bl
                nc.vector.tensor_add(
                    onat[:, n0:n0 + 4, :], wn[:],
                    b2t.unsqueeze(1).broadcast_to([128, 4, INPUT_DIM]))
                outr = out.rearrange("(p n) d -> p n d", n=NCHUNK)
                if g == NCTILE - 1:
                    # last ctile: per-block DMA on hwdge (gpsimd's software
                    # DGE issue + end-of-program drain sat on the tail)
                    nc.scalar.dma_start(outr[:, n0:n0 + 4, :],
                                        onat[:, n0:n0 + 4, :])
                elif bl == 3:
                    nc.scalar.dma_start(
                        outr[:, 16 * g:16 * (g + 1), :],
                        onat[:, 16 * g:16 * (g + 1), :])

            # ctile 0 staged in 4-chunk pieces for a short lead-in; later
            # ctiles staged whole, one ctile ahead of the PE.
            stage_tree(0, 4)
            stage_transpose(0, 4)
            # Square table loads here, off the first-piece critical chain
            nc.scalar.activation(warm[:, 1:2], zero[0:1, :], AFT.Square)
            SCHED = {0: (4, 4), 1: (8, 4), 2: (12, 4),
                     3: (16, 16), 6: (32, 16), 10: (48, 16)}
            for i in range(NBLK + 2):
                if i < NBLK:
                    emit_phi(i)
                    if i in SCHED:
                        stage_tree(*SCHED[i])
                        stage_transpose(*SCHED[i])
                if 1 <= i <= NBLK:
                    emit_contract(i - 1)
                if i >= 2:
                    emit_head(i - 2)

    nc.compile()
    return nc


_NC_CACHE = []


def _get_nc():
    if not _NC_CACHE:
        _NC_CACHE.append(_build_nc())
    return _NC_CACHE[0]


def kernel(x, q_weights, w1, b1, w2, b2):
    global LAST_RESULTS
    x = np.ascontiguousarray(np.asarray(x, dtype=np.float32))
    consts = _host_consts(np.asarray(q_weights), np.asarray(w1),
                          np.asarray(b1), np.asarray(w2), np.asarray(b2))
    nc = _get_nc()
    in_maps = [
        {'xs': np.ascontiguousarray(x[i * BC:(i + 1) * BC]), **consts}
        for i in range(NCORES)
    ]
    res = run_bass_kernel_spmd(nc, in_maps, list(range(NCORES)))
    LAST_RESULTS = res
    return np.concatenate([res.results[i]['out'] for i in range(NCORES)],
                          axis=0).astype(np.float32)
